# revision 11
# baseline (speedup 1.0000x reference)
"""CenterLoss (segment_reduce) Trainium2 Bass kernel — fp8 DoubleRow rewrite.

loss·N = t1 + t2 - 2·t3 with
  t1 = sum_i fsq[i]·rowcnt[i],  t2 = sum_c csq[c]·colcnt[c],
  t3 = sum_{c,f} Z[c,f]·centers[c,f],  Z = mask^T @ features.

Everything is folded into ONE device contraction Z2 = X^T @ mask with the
augmented X = [features | 1 | (fsq-256)/2] (258 cols, fp8) and an epilogue
elementwise reduce against W = [-2·centers^T ; csq+256 ; 2]:
  sum(Z2 ∘ W) = -2·t3 + (t2 + 256·T0) + (t1 - 256·T0) = N·loss_partial.
(The fsq column is mean-centred so its fp8 quantisation error is ~0.8% of
fsq instead of ~6%; the 256·T0 cross-terms cancel via the csq+256 row.)

Per core (8-way data-parallel on rows):
  - mask staged host-side as fp8 bytes (0.0/1.0 exact): 1 B/elt instead of
    the baseline's 4 B/elt int32 — 4x less HBM traffic on the dominant
    stream (8.26 MB/core vs 32.8 MB).
  - X is the STATIONARY matmul operand (3 chunks of 128|128|2 cols,
    weights reused across class halves so LDWEIGHTS hides), the mask
    STREAMS through the PE.  perf_mode=DoubleRow contracts 256 rows per
    pass (2 fp8 MACs/cell/cycle) — 32 double-tiles of 6 matmuls each,
    accumulating in 6 PSUM banks (class halves 512|488 to stay within the
    2 KB bank limit).
  - Epilogue: 6 DVE scalar_tensor_tensor mult+accum ops against the
    staged W -> one [128, 6] partial per core; host sums in f64 and
    divides by N (the all-reduce of the sharding hint).
All tensors are staged host-side in the exact [partition, tile, col] SBUF
layout, so every DMA is a contiguous per-partition HWDGE copy (no casts,
no rearrange descriptors).
"""

import numpy as np

N_TOTAL = 65536
C = 1000
F = 256
NCORES = 8
NSH = N_TOTAL // NCORES  # 8192 rows per core
P = 128                  # partitions (rows per k-tile)
T = NSH // P             # 64 row tiles per core
TD = T // 2              # 32 DoubleRow tiles (256 rows each)
CP = 1008                # class dim padded to %16 for DR access patterns
FP = 272                 # featx col dim padded to %16 (258 used)
FS = F + 2               # used featx cols: features | ones | fsq-resid
H0, H1 = 512, C - 512    # class halves (PSUM bank = 512 f32)


NWARM = 24  # cold N=128 warmup matmuls ≈ 3.2 µs of PE activity (HAM window)


def build_bass(fix_waits=True):
    import concourse.bass as bass
    import concourse.mybir as mybir
    import concourse.tile as tile
    from contextlib import ExitStack

    f32 = mybir.dt.float32
    bf16 = mybir.dt.bfloat16
    f8 = mybir.dt.float8e4
    DR = mybir.MatmulPerfMode.DoubleRow
    bypass = mybir.AluOpType.bypass
    mult = mybir.AluOpType.mult

    nc = bass.Bass(trn_type="TRN2")
    mask_d = nc.dram_tensor("mask", [P, T, CP], f8, kind="ExternalInput")
    featx_d = nc.dram_tensor("featx", [P, T, FP], f8, kind="ExternalInput")
    centw_d = nc.dram_tensor("centw", [P, 2, C], bf16, kind="ExternalInput")
    cento_d = nc.dram_tensor("cento", [2, C], f32, kind="ExternalInput")
    out_d = nc.dram_tensor("partial", [P, 6], f32, kind="ExternalOutput")

    with tile.TileContext(nc) as tc, ExitStack() as ctx:
        const = ctx.enter_context(tc.tile_pool(name="const", bufs=1))
        zp = ctx.enter_context(tc.tile_pool(name="zp", bufs=1, space="PSUM"))

        # fully resident SBUF tensors (nothing recycles)
        mask_full = const.tile([P, T, CP], f8, name="mask_full")
        featx_full = const.tile([P, T, FP], f8, name="featx_full")
        centw = const.tile([P, 2, C], bf16, name="centw")
        cento = const.tile([2, C], f32, name="cento")
        acc = const.tile([P, 6], f32, name="acc")
        junk = const.tile([P, H0], bf16, name="junk")
        obs = const.tile([1, 2], f32, name="obs")
        zw = const.tile([P, 128], f8, name="zw")

        # 7 PSUM banks: 4 feature-chunk accumulators + 2 ones/fsq rows
        # + 1 warmup scratch
        zf = zp.tile([P, 4, 512], f32, name="zf")
        zo = zp.tile([P, 2, 512], f32, name="zo")
        psw = zp.tile([P, 512], f32, name="psw")

        nc.vector.memset(acc, 0.0)
        nc.vector.memset(zw, 0.0)

        def mask_dma(a, b):
            nc.sync.dma_start(out=mask_full[:, a:b, :],
                              in_=mask_d[:, a:b, :])

        def featx_dma(a, b):
            nc.sync.dma_start(out=featx_full[:, a:b, :],
                              in_=featx_d[:, a:b, :])

        # One FIFO HWDGE queue: featx front-loaded between the first mask
        # tiles, epilogue weights after mask 8; mask j's completion implies
        # everything emitted before it has landed.
        featx_dma(0, 2)
        mask_dma(0, 2)
        featx_dma(2, 8)
        mask_dma(2, 4)
        featx_dma(8, 24)
        mask_dma(4, 6)
        mask_dma(6, 8)
        featx_dma(24, 44)
        mask_dma(8, 10)
        mask_dma(10, 12)
        featx_dma(44, 64)
        mask_dma(12, 14)
        mask_dma(14, 16)
        nc.sync.dma_start(out=centw, in_=centw_d[:, :, :])
        nc.sync.dma_start(out=cento, in_=cento_d[:, :])
        # chained 1-element DVE reads: DVE observes the epilogue-weight
        # DMAs here, so each epilogue STT later needs only its PE wait
        # (walrus encodes a limited number of sync waits per STT).
        nc.vector.tensor_copy(out=obs[0:1, 0:1], in_=centw[0:1, 0, 0:1])
        nc.vector.tensor_copy(out=obs[0:1, 1:2], in_=cento[0:1, 0:1])
        for t in range(16, T, 4):
            mask_dma(t, t + 4)

        # warmup matmuls on zeros: trip the PE HAM clock gate (~3.4 µs of
        # sustained activity) during the startup DMA window so the real
        # stream runs at 2.4 GHz from its first tile.
        for _ in range(NWARM):
            nc.tensor.matmul(psw[:, 0:128], lhsT=zw[:, 0:128],
                             rhs=zw[:, 0:128], start=True, stop=True)

        # Main stream: per-bank-group staggered tile order.  Group A
        # (feature cols 0:128) processes double-tile r in round r, group B
        # (cols 128:256) tile r-2, group O (ones/fsq) tile r-4.  Each
        # group's accumulation therefore STOPS two rounds apart, letting
        # the epilogue STTs pipeline with the thinning matmul tail instead
        # of serializing after it (and the thin early rounds ease the
        # startup DMA race).
        def mm(bank, j, lhs_lo, lhs_hi, half):
            lhs = featx_full[:, 2 * j:2 * j + 2, lhs_lo:lhs_hi]
            if half == 0:
                rhs = mask_full[:, 2 * j:2 * j + 2, 0:H0]
            else:
                rhs = mask_full[:, 2 * j:2 * j + 2, H0:C]
            n = H0 if half == 0 else H1
            if bank < 4:
                out = zf[:, bank, 0:n]
            else:
                out = zo[0:2, bank - 4, 0:n]
            nc.tensor.matmul(out, lhsT=lhs, rhs=rhs,
                             start=(j == 0), stop=(j == TD - 1),
                             perf_mode=DR)

        for r in range(TD + 4):
            if r < TD:
                mm(0, r, 0, 128, 0)
                mm(1, r, 0, 128, 1)
            if 0 <= r - 2 < TD:
                mm(2, r - 2, 128, 256, 0)
                mm(3, r - 2, 128, 256, 1)
            if 0 <= r - 4 < TD:
                mm(4, r - 4, 256, 258, 0)
                mm(5, r - 4, 256, 258, 1)

        # ---- epilogue: fused mul+reduce of Z2 against the staged W ----
        def stt(i0, i1, slot, n, parts=P):
            nc.vector.scalar_tensor_tensor(
                out=junk[0:parts, 0:n],
                in0=i0,
                scalar=1.0,
                in1=i1,
                op0=bypass,
                op1=mult,
                accum_out=acc[0:parts, slot:slot + 1],
            )

        stt(zf[:, 0, 0:H0], centw[:, 0, 0:H0], 0, H0)
        stt(zf[:, 1, 0:H1], centw[:, 0, H0:C], 1, H1)
        stt(zf[:, 2, 0:H0], centw[:, 1, 0:H0], 2, H0)
        stt(zf[:, 3, 0:H1], centw[:, 1, H0:C], 3, H1)
        stt(zo[0:2, 0, 0:H0], cento[0:2, 0:H0], 4, H0, parts=2)
        stt(zo[0:2, 1, 0:H1], cento[0:2, H0:C], 5, H1, parts=2)

        nc.sync.dma_start(out=out_d[:, :], in_=acc)

    if fix_waits:
        _fix_sync_waits(nc)
    return nc


def _fix_sync_waits(nc):
    """Strip provably-redundant same-engine semaphore self-waits.

    Tile encodes some cross-instruction deps as waits on the instruction's
    own engine semaphore at a value already reached by an EARLIER
    instruction on the same (in-order) engine — trivially satisfied by
    program order.  Walrus can only encode one sync wait on an STT, so
    these must go.  Every remaining compute instruction must have <=1
    wait (drains may keep several; walrus accepts that).
    """
    insts = []
    for f in nc.m.functions:
        for b in f.blocks:
            insts.extend(b.instructions)

    # which engines increment each semaphore
    updaters = {}
    out_sems = set()
    for inst in insts:
        si = inst.sync_info
        if si is None:
            continue
        for u in si.on_update:
            updaters.setdefault(u.ant_name, set()).add(inst.engine)
        if (type(inst).__name__ == "InstDMACopy" and inst.outs
                and str(inst.outs[0].memsetref).startswith("partial")):
            for u in si.on_update:
                out_sems.add(u.ant_name)
    assert out_sems, "no output DMA found"

    # cumulative per-engine increments in program order
    cum = {}
    for inst in insts:
        si = inst.sync_info
        tn = type(inst).__name__
        if si is None:
            continue
        if tn == "InstDrain" and len(si.on_wait) > 1:
            # kernel-tail drains only need the DRAM-output DMA's sem:
            # every input DMA's completion is implied by its consumers,
            # which the per-engine drains already order after.
            keep = [w for w in si.on_wait if w.ant_name in out_sems]
            assert keep, (
                f"drain {inst.name}: no output-DMA wait among "
                f"{[w.ant_name for w in si.on_wait]}")
            inst.sync_info = type(si)(on_wait=keep, on_update=si.on_update)
            si = inst.sync_info
        elif tn != "InstDrain" and len(si.on_wait) > 1:
            keep = []
            for w in si.on_wait:
                eng_cnt = cum.get((inst.engine, w.ant_name), 0)
                same_engine_only = updaters.get(w.ant_name) == {inst.engine}
                if (same_engine_only and w.wait_value is not None
                        and eng_cnt >= w.wait_value):
                    continue  # satisfied by in-order execution
                keep.append(w)
            assert len(keep) <= 1, (
                f"{tn} {inst.name} ({inst.engine}): still multi-wait "
                f"{[(w.ant_name, w.wait_value) for w in keep]}")
            inst.sync_info = type(si)(on_wait=keep, on_update=si.on_update)
            si = inst.sync_info
        for u in si.on_update:
            key = (inst.engine, u.ant_name)
            cum[key] = cum.get(key, 0) + (u.update_value or 1)


def _shard_inputs(inputs):
    import ml_dtypes

    fp8 = ml_dtypes.float8_e4m3
    gt = np.asarray(inputs["gt"])
    features = np.asarray(inputs["features"], dtype=np.float32)
    centers = np.asarray(inputs["centers"], dtype=np.float32)

    # mask: fp8 bytes, exactly 0.0 / 1.0 (0x00 / 0x38), laid out
    # [core, p, t, c] with the class dim zero-padded to CP.
    m8 = (gt != 0).astype(np.uint8) * np.uint8(0x38)
    mask_st = np.zeros((NCORES, P, T, CP), dtype=np.uint8)
    mask_st[..., :C] = m8.reshape(NCORES, T, P, C).transpose(0, 2, 1, 3)
    mask_st = mask_st.view(fp8)

    # featx: [features | 1 | (fsq-256)/2] in fp8, same layout, padded to FP
    fsq = (features.astype(np.float64) ** 2).sum(axis=1)
    fx = np.empty((N_TOTAL, FS), dtype=np.float32)
    fx[:, 0:F] = features
    fx[:, F] = 1.0
    fx[:, F + 1] = (fsq - 256.0) * 0.5
    fx8 = fx.astype(fp8)
    featx_st = np.zeros((NCORES, P, T, FP), dtype=fp8)
    featx_st[..., :FS] = fx8.reshape(NCORES, T, P, FS).transpose(0, 2, 1, 3)

    # epilogue weights (replicated): centw[p, k, c] = -2*centers[c, 128k+p]
    centw = np.ascontiguousarray(
        (-2.0 * centers.T).reshape(2, P, C).transpose(1, 0, 2)
    ).astype(ml_dtypes.bfloat16)
    csq = (centers.astype(np.float64) ** 2).sum(axis=1)
    cento = np.empty((2, C), dtype=np.float32)
    cento[0] = csq + 256.0
    cento[1] = 2.0

    in_maps = []
    for c in range(NCORES):
        in_maps.append({
            "mask": np.ascontiguousarray(mask_st[c]),
            "featx": np.ascontiguousarray(featx_st[c]),
            "centw": centw,
            "cento": cento,
        })
    return in_maps


def _combine(results):
    """Host-side scalar combine (the all-reduce of the sharding hint).

    Per-core partial [128, 6]: cols 0-3 are per-partition sums of
    Z2_feat ∘ (-2 centers^T) (= -2·t3), cols 4-5 are valid on partitions
    0-1 only: colcnt·(csq+256) and 2·fsq-resid sums (= t1 + t2).
    """
    total = 0.0
    for r in results:
        part = np.asarray(r["partial"], dtype=np.float64)
        total += part[:, 0:4].sum() + part[0:2, 4:6].sum()
    return total / N_TOTAL


def run_spmd(inputs, trace=False):
    """Compile + run on all 8 cores. Returns (loss_scalar, BassKernelResults)."""
    from concourse.bass_utils import run_bass_kernel_spmd

    nc = build_bass()
    in_maps = _shard_inputs(inputs)
    res = run_bass_kernel_spmd(
        nc, in_maps, core_ids=list(range(NCORES)), trace=trace,
    )
    loss = _combine(res.results)
    return np.array(np.float32(loss), dtype=np.float32), res


def kernel(**inputs):
    loss, _ = run_spmd(inputs, trace=False)
    return loss


if __name__ == "__main__":
    # quick CoreSim numerical check on core 0's shard
    from concourse.bass_interp import CoreSim

    rng = np.random.default_rng(0)
    gt = (rng.integers(0, 2, size=(N_TOTAL, C))).astype(np.int32)
    features = rng.standard_normal((N_TOTAL, F)).astype(np.float32)
    centers = rng.standard_normal((C, F)).astype(np.float32)

    in_maps = _shard_inputs({"gt": gt, "features": features,
                             "centers": centers})

    nc = build_bass(fix_waits=False)
    sim = CoreSim(nc, require_finite=True, require_nnan=True)
    for k, v in in_maps[0].items():
        sim.tensor(k)[:] = v
    sim.simulate()

    got = _combine([{"partial": np.asarray(sim.tensor("partial"))}]) * N_TOTAL

    sl = slice(0, NSH)
    mask = (gt[sl] > 0).astype(np.float64)
    f64 = features[sl].astype(np.float64)
    c64 = centers.astype(np.float64)
    dist = (
        (f64 * f64).sum(1)[:, None]
        + (c64 * c64).sum(1)[None, :]
        - 2.0 * (f64 @ c64.T)
    )
    want = float((mask * dist).sum())
    print(f"sim partial sum = {got:.6e}  want = {want:.6e}  "
          f"rel = {abs(got - want) / abs(want):.3e}")


# revision 15
# speedup vs baseline: 1.0299x; 1.0299x over previous
"""CenterLoss (segment_reduce) Trainium2 Bass kernel — fp8 DoubleRow rewrite.

loss·N = t1 + t2 - 2·t3 with
  t1 = sum_i fsq[i]·rowcnt[i],  t2 = sum_c csq[c]·colcnt[c],
  t3 = sum_{c,f} Z[c,f]·centers[c,f],  Z = mask^T @ features.

Everything is folded into ONE device contraction Z2 = X^T @ mask with the
augmented X = [features | 1 | (fsq-256)/2] (258 cols, fp8) and an epilogue
elementwise reduce against W = [-2·centers^T ; csq+256 ; 2]:
  sum(Z2 ∘ W) = -2·t3 + (t2 + 256·T0) + (t1 - 256·T0) = N·loss_partial.
(The fsq column is mean-centred so its fp8 quantisation error is ~0.8% of
fsq instead of ~6%; the 256·T0 cross-terms cancel via the csq+256 row.)

Per core (8-way data-parallel on rows):
  - mask staged host-side as fp8 bytes (0.0/1.0 exact): 1 B/elt instead of
    the baseline's 4 B/elt int32 — 4x less HBM traffic on the dominant
    stream (8.26 MB/core vs 32.8 MB).
  - X is the STATIONARY matmul operand (3 chunks of 128|128|2 cols,
    weights reused across class halves so LDWEIGHTS hides), the mask
    STREAMS through the PE.  perf_mode=DoubleRow contracts 256 rows per
    pass (2 fp8 MACs/cell/cycle) — 32 double-tiles of 6 matmuls each,
    accumulating in 6 PSUM banks (class halves 512|488 to stay within the
    2 KB bank limit).
  - Epilogue: 6 DVE scalar_tensor_tensor mult+accum ops against the
    staged W -> one [128, 6] partial per core; host sums in f64 and
    divides by N (the all-reduce of the sharding hint).
All tensors are staged host-side in the exact [partition, tile, col] SBUF
layout, so every DMA is a contiguous per-partition HWDGE copy (no casts,
no rearrange descriptors).
"""

import numpy as np

N_TOTAL = 65536
C = 1000
F = 256
NCORES = 8
NSH = N_TOTAL // NCORES  # 8192 rows per core
P = 128                  # partitions (rows per k-tile)
T = NSH // P             # 64 row tiles per core
TD = T // 2              # 32 DoubleRow tiles (256 rows each)
CP = 1008                # class dim padded to %16 for DR access patterns
FP = 272                 # featx col dim padded to %16 (258 used)
FS = F + 2               # used featx cols: features | ones | fsq-resid
H0, H1 = 512, C - 512    # class halves (PSUM bank = 512 f32)


NWARM = 34  # cold N=128 warmup matmuls ≈ 3.6 µs of PE activity: spans the
            # 8..11.5 µs window between PE release and first mask tile so a
            # full HAM busy-window completes right as the real stream starts


def build_bass(fix_waits=True):
    import concourse.bass as bass
    import concourse.mybir as mybir
    import concourse.tile as tile
    from contextlib import ExitStack

    f32 = mybir.dt.float32
    bf16 = mybir.dt.bfloat16
    f8 = mybir.dt.float8e4
    DR = mybir.MatmulPerfMode.DoubleRow
    bypass = mybir.AluOpType.bypass
    mult = mybir.AluOpType.mult

    nc = bass.Bass(trn_type="TRN2")
    mask_d = nc.dram_tensor("mask", [P, T, CP], f8, kind="ExternalInput")
    featx_d = nc.dram_tensor("featx", [P, T, FP], f8, kind="ExternalInput")
    centw_d = nc.dram_tensor("centw", [P, 2, C], bf16, kind="ExternalInput")
    cento_d = nc.dram_tensor("cento", [2, C], f32, kind="ExternalInput")
    out_d = nc.dram_tensor("partial", [P, 6], f32, kind="ExternalOutput")

    with tile.TileContext(nc) as tc, ExitStack() as ctx:
        const = ctx.enter_context(tc.tile_pool(name="const", bufs=1))
        zp = ctx.enter_context(tc.tile_pool(name="zp", bufs=1, space="PSUM"))

        # fully resident SBUF tensors (nothing recycles)
        mask_full = const.tile([P, T, CP], f8, name="mask_full")
        featx_full = const.tile([P, T, FP], f8, name="featx_full")
        centw = const.tile([P, 2, C], bf16, name="centw")
        cento = const.tile([2, C], f32, name="cento")
        acc = const.tile([P, 6], f32, name="acc")
        junk = const.tile([P, H0], bf16, name="junk")
        obs = const.tile([1, 2], f32, name="obs")
        zw = const.tile([P, 128], f8, name="zw")

        # 7 PSUM banks: 4 feature-chunk accumulators + 2 ones/fsq rows
        # + 1 warmup scratch.  One tile PER BANK so Tile's dependency
        # tracking pins each epilogue STT to exactly its bank's stop-MM.
        zb = [zp.tile([P, 512], f32, name=f"zb{k}") for k in range(6)]
        psw = zp.tile([P, 512], f32, name="psw")

        nc.vector.memset(acc, 0.0)
        nc.vector.memset(zw, 0.0)

        def mask_dma(a, b):
            nc.sync.dma_start(out=mask_full[:, a:b, :],
                              in_=mask_d[:, a:b, :])

        def featx_dma(a, b):
            nc.sync.dma_start(out=featx_full[:, a:b, :],
                              in_=featx_d[:, a:b, :])

        # One FIFO HWDGE queue: featx front-loaded between the first mask
        # tiles, epilogue weights after mask 8; mask j's completion implies
        # everything emitted before it has landed.
        featx_dma(0, 2)
        mask_dma(0, 2)
        featx_dma(2, 8)
        mask_dma(2, 4)
        featx_dma(8, 24)
        mask_dma(4, 6)
        mask_dma(6, 8)
        featx_dma(24, 44)
        mask_dma(8, 10)
        mask_dma(10, 12)
        featx_dma(44, 64)
        mask_dma(12, 14)
        mask_dma(14, 16)
        nc.sync.dma_start(out=centw, in_=centw_d[:, :, :])
        nc.sync.dma_start(out=cento, in_=cento_d[:, :])
        # chained 1-element DVE reads: DVE observes the epilogue-weight
        # DMAs here, so each epilogue STT later needs only its PE wait
        # (walrus encodes a limited number of sync waits per STT).
        nc.vector.tensor_copy(out=obs[0:1, 0:1], in_=centw[0:1, 0, 0:1])
        nc.vector.tensor_copy(out=obs[0:1, 1:2], in_=cento[0:1, 0:1])
        for t in range(16, T, 4):
            mask_dma(t, t + 4)

        # warmup matmuls on zeros: trip the PE HAM clock gate (~3.4 µs of
        # sustained activity) during the startup DMA window so the real
        # stream runs at 2.4 GHz from its first tile.
        for _ in range(NWARM):
            nc.tensor.matmul(psw[:, 0:128], lhsT=zw[:, 0:128],
                             rhs=zw[:, 0:128], start=True, stop=True)

        # Main stream: per-bank-group staggered tile order.  Group A
        # (feature cols 0:128) processes double-tile r in round r, group B
        # (cols 128:256) tile r-2, group O (ones/fsq) tile r-4.  Each
        # group's accumulation therefore STOPS two rounds apart, letting
        # the epilogue STTs pipeline with the thinning matmul tail instead
        # of serializing after it (and the thin early rounds ease the
        # startup DMA race).
        def mm(bank, j, lhs_lo, lhs_hi, half):
            lhs = featx_full[:, 2 * j:2 * j + 2, lhs_lo:lhs_hi]
            if half == 0:
                rhs = mask_full[:, 2 * j:2 * j + 2, 0:H0]
            else:
                rhs = mask_full[:, 2 * j:2 * j + 2, H0:C]
            n = H0 if half == 0 else H1
            if bank < 4:
                out = zb[bank][:, 0:n]
            else:
                out = zb[bank][0:2, 0:n]
            nc.tensor.matmul(out, lhsT=lhs, rhs=rhs,
                             start=(j == 0), stop=(j == TD - 1),
                             perf_mode=DR)

        for r in range(TD + 4):
            if r < TD:
                mm(0, r, 0, 128, 0)
                mm(1, r, 0, 128, 1)
            if 0 <= r - 2 < TD:
                mm(2, r - 2, 128, 256, 0)
                mm(3, r - 2, 128, 256, 1)
            if 0 <= r - 4 < TD:
                mm(4, r - 4, 256, 258, 0)
                mm(5, r - 4, 256, 258, 1)

        # ---- epilogue: fused mul+reduce of Z2 against the staged W ----
        def stt(i0, i1, slot, n, parts=P):
            nc.vector.scalar_tensor_tensor(
                out=junk[0:parts, 0:n],
                in0=i0,
                scalar=1.0,
                in1=i1,
                op0=bypass,
                op1=mult,
                accum_out=acc[0:parts, slot:slot + 1],
            )

        stt(zb[0][:, 0:H0], centw[:, 0, 0:H0], 0, H0)
        stt(zb[1][:, 0:H1], centw[:, 0, H0:C], 1, H1)
        stt(zb[2][:, 0:H0], centw[:, 1, 0:H0], 2, H0)
        stt(zb[3][:, 0:H1], centw[:, 1, H0:C], 3, H1)
        stt(zb[4][0:2, 0:H0], cento[0:2, 0:H0], 4, H0, parts=2)
        stt(zb[5][0:2, 0:H1], cento[0:2, H0:C], 5, H1, parts=2)

        nc.sync.dma_start(out=out_d[:, :], in_=acc)

    if fix_waits:
        _fix_sync_waits(nc)
    return nc


def _fix_sync_waits(nc):
    """Strip provably-redundant same-engine semaphore self-waits.

    Tile encodes some cross-instruction deps as waits on the instruction's
    own engine semaphore at a value already reached by an EARLIER
    instruction on the same (in-order) engine — trivially satisfied by
    program order.  Walrus can only encode one sync wait on an STT, so
    these must go.  Every remaining compute instruction must have <=1
    wait (drains may keep several; walrus accepts that).
    """
    insts = []
    for f in nc.m.functions:
        for b in f.blocks:
            insts.extend(b.instructions)

    # which engines increment each semaphore
    updaters = {}
    out_sems = set()
    for inst in insts:
        si = inst.sync_info
        if si is None:
            continue
        for u in si.on_update:
            updaters.setdefault(u.ant_name, set()).add(inst.engine)
        if (type(inst).__name__ == "InstDMACopy" and inst.outs
                and str(inst.outs[0].memsetref).startswith("partial")):
            for u in si.on_update:
                out_sems.add(u.ant_name)
    assert out_sems, "no output DMA found"

    # cumulative per-engine increments in program order
    cum = {}
    for inst in insts:
        si = inst.sync_info
        tn = type(inst).__name__
        if si is None:
            continue
        if tn == "InstDrain" and len(si.on_wait) > 1:
            # kernel-tail drains only need the DRAM-output DMA's sem:
            # every input DMA's completion is implied by its consumers,
            # which the per-engine drains already order after.
            keep = [w for w in si.on_wait if w.ant_name in out_sems]
            assert keep, (
                f"drain {inst.name}: no output-DMA wait among "
                f"{[w.ant_name for w in si.on_wait]}")
            inst.sync_info = type(si)(on_wait=keep, on_update=si.on_update)
            si = inst.sync_info
        elif tn != "InstDrain" and len(si.on_wait) > 1:
            keep = []
            for w in si.on_wait:
                eng_cnt = cum.get((inst.engine, w.ant_name), 0)
                same_engine_only = updaters.get(w.ant_name) == {inst.engine}
                if (same_engine_only and w.wait_value is not None
                        and eng_cnt >= w.wait_value):
                    continue  # satisfied by in-order execution
                keep.append(w)
            assert len(keep) <= 1, (
                f"{tn} {inst.name} ({inst.engine}): still multi-wait "
                f"{[(w.ant_name, w.wait_value) for w in keep]}")
            inst.sync_info = type(si)(on_wait=keep, on_update=si.on_update)
            si = inst.sync_info
        for u in si.on_update:
            key = (inst.engine, u.ant_name)
            cum[key] = cum.get(key, 0) + (u.update_value or 1)


def _shard_inputs(inputs):
    import ml_dtypes

    fp8 = ml_dtypes.float8_e4m3
    gt = np.asarray(inputs["gt"])
    features = np.asarray(inputs["features"], dtype=np.float32)
    centers = np.asarray(inputs["centers"], dtype=np.float32)

    # mask: fp8 bytes, exactly 0.0 / 1.0 (0x00 / 0x38), laid out
    # [core, p, t, c] with the class dim zero-padded to CP.
    m8 = (gt != 0).astype(np.uint8) * np.uint8(0x38)
    mask_st = np.zeros((NCORES, P, T, CP), dtype=np.uint8)
    mask_st[..., :C] = m8.reshape(NCORES, T, P, C).transpose(0, 2, 1, 3)
    mask_st = mask_st.view(fp8)

    # featx: [features | 1 | (fsq-256)/2] in fp8, same layout, padded to FP
    fsq = (features.astype(np.float64) ** 2).sum(axis=1)
    fx = np.empty((N_TOTAL, FS), dtype=np.float32)
    fx[:, 0:F] = features
    fx[:, F] = 1.0
    fx[:, F + 1] = (fsq - 256.0) * 0.5
    fx8 = fx.astype(fp8)
    featx_st = np.zeros((NCORES, P, T, FP), dtype=fp8)
    featx_st[..., :FS] = fx8.reshape(NCORES, T, P, FS).transpose(0, 2, 1, 3)

    # epilogue weights (replicated): centw[p, k, c] = -2*centers[c, 128k+p]
    centw = np.ascontiguousarray(
        (-2.0 * centers.T).reshape(2, P, C).transpose(1, 0, 2)
    ).astype(ml_dtypes.bfloat16)
    csq = (centers.astype(np.float64) ** 2).sum(axis=1)
    cento = np.empty((2, C), dtype=np.float32)
    cento[0] = csq + 256.0
    cento[1] = 2.0

    in_maps = []
    for c in range(NCORES):
        in_maps.append({
            "mask": np.ascontiguousarray(mask_st[c]),
            "featx": np.ascontiguousarray(featx_st[c]),
            "centw": centw,
            "cento": cento,
        })
    return in_maps


def _combine(results):
    """Host-side scalar combine (the all-reduce of the sharding hint).

    Per-core partial [128, 6]: cols 0-3 are per-partition sums of
    Z2_feat ∘ (-2 centers^T) (= -2·t3), cols 4-5 are valid on partitions
    0-1 only: colcnt·(csq+256) and 2·fsq-resid sums (= t1 + t2).
    """
    total = 0.0
    for r in results:
        part = np.asarray(r["partial"], dtype=np.float64)
        total += part[:, 0:4].sum() + part[0:2, 4:6].sum()
    return total / N_TOTAL


def run_spmd(inputs, trace=False):
    """Compile + run on all 8 cores. Returns (loss_scalar, BassKernelResults)."""
    from concourse.bass_utils import run_bass_kernel_spmd

    nc = build_bass()
    in_maps = _shard_inputs(inputs)
    res = run_bass_kernel_spmd(
        nc, in_maps, core_ids=list(range(NCORES)), trace=trace,
    )
    loss = _combine(res.results)
    return np.array(np.float32(loss), dtype=np.float32), res


def kernel(**inputs):
    loss, _ = run_spmd(inputs, trace=False)
    return loss


if __name__ == "__main__":
    # quick CoreSim numerical check on core 0's shard
    from concourse.bass_interp import CoreSim

    rng = np.random.default_rng(0)
    gt = (rng.integers(0, 2, size=(N_TOTAL, C))).astype(np.int32)
    features = rng.standard_normal((N_TOTAL, F)).astype(np.float32)
    centers = rng.standard_normal((C, F)).astype(np.float32)

    in_maps = _shard_inputs({"gt": gt, "features": features,
                             "centers": centers})

    nc = build_bass(fix_waits=False)
    sim = CoreSim(nc, require_finite=True, require_nnan=True)
    for k, v in in_maps[0].items():
        sim.tensor(k)[:] = v
    sim.simulate()

    got = _combine([{"partial": np.asarray(sim.tensor("partial"))}]) * N_TOTAL

    sl = slice(0, NSH)
    mask = (gt[sl] > 0).astype(np.float64)
    f64 = features[sl].astype(np.float64)
    c64 = centers.astype(np.float64)
    dist = (
        (f64 * f64).sum(1)[:, None]
        + (c64 * c64).sum(1)[None, :]
        - 2.0 * (f64 @ c64.T)
    )
    want = float((mask * dist).sum())
    print(f"sim partial sum = {got:.6e}  want = {want:.6e}  "
          f"rel = {abs(got - want) / abs(want):.3e}")


# revision 18
# speedup vs baseline: 1.1108x; 1.0786x over previous
"""CenterLoss (segment_reduce) Trainium2 Bass kernel — fp8 DoubleRow rewrite.

loss·N = t1 + t2 - 2·t3 with
  t1 = sum_i fsq[i]·rowcnt[i],  t2 = sum_c csq[c]·colcnt[c],
  t3 = sum_{c,f} Z[c,f]·centers[c,f],  Z = mask^T @ features.

Everything is folded into ONE device contraction Z2 = X^T @ mask with the
augmented X = [features | 1 | (fsq-256)/2] (258 cols, fp8) and an epilogue
elementwise reduce against W = [-2·centers^T ; csq+256 ; 2]:
  sum(Z2 ∘ W) = -2·t3 + (t2 + 256·T0) + (t1 - 256·T0) = N·loss_partial.
(The fsq column is mean-centred so its fp8 quantisation error is ~0.8% of
fsq instead of ~6%; the 256·T0 cross-terms cancel via the csq+256 row.)

Per core (8-way data-parallel on rows):
  - mask staged host-side as fp8 bytes (0.0/1.0 exact): 1 B/elt instead of
    the baseline's 4 B/elt int32 — 4x less HBM traffic on the dominant
    stream (8.26 MB/core vs 32.8 MB).
  - X is the STATIONARY matmul operand (3 chunks of 128|128|2 cols,
    weights reused across class halves so LDWEIGHTS hides), the mask
    STREAMS through the PE.  perf_mode=DoubleRow contracts 256 rows per
    pass (2 fp8 MACs/cell/cycle) — 32 double-tiles of 6 matmuls each,
    accumulating in 6 PSUM banks (class halves 512|488 to stay within the
    2 KB bank limit).
  - Epilogue: 6 DVE scalar_tensor_tensor mult+accum ops against the
    staged W -> one [128, 6] partial per core; host sums in f64 and
    divides by N (the all-reduce of the sharding hint).
All tensors are staged host-side in the exact [partition, tile, col] SBUF
layout, so every DMA is a contiguous per-partition HWDGE copy (no casts,
no rearrange descriptors).
"""

import numpy as np

N_TOTAL = 65536
C = 1000
F = 256
NCORES = 8
NSH = N_TOTAL // NCORES  # 8192 rows per core
P = 128                  # partitions (rows per k-tile)
T = NSH // P             # 64 row tiles per core
TD = T // 2              # 32 DoubleRow tiles (256 rows each)
CP = 1008                # class dim padded to %16 for DR access patterns
FP = 272                 # featx col dim padded to %16 (258 used)
FS = F + 2               # used featx cols: features | ones | fsq-resid
H0, H1 = 512, C - 512    # class halves (PSUM bank = 512 f32)


NWARM = 34  # cold N=128 warmup matmuls ≈ 3.6 µs of PE activity: spans the
            # 8..11.5 µs window between PE release and first mask tile so a
            # full HAM busy-window completes right as the real stream starts


def build_bass(fix_waits=True):
    import concourse.bass as bass
    import concourse.mybir as mybir
    import concourse.tile as tile
    from contextlib import ExitStack

    f32 = mybir.dt.float32
    bf16 = mybir.dt.bfloat16
    f8 = mybir.dt.float8e4
    DR = mybir.MatmulPerfMode.DoubleRow
    bypass = mybir.AluOpType.bypass
    mult = mybir.AluOpType.mult

    nc = bass.Bass(trn_type="TRN2")
    mask_d = nc.dram_tensor("mask", [P, T, CP], f8, kind="ExternalInput")
    featx_d = nc.dram_tensor("featx", [P, T, FP], f8, kind="ExternalInput")
    centw_d = nc.dram_tensor("centw", [P, 2, C], bf16, kind="ExternalInput")
    cento_d = nc.dram_tensor("cento", [2, C], f32, kind="ExternalInput")
    out_d = nc.dram_tensor("partial", [P, 6], f32, kind="ExternalOutput")

    with tile.TileContext(nc) as tc, ExitStack() as ctx:
        const = ctx.enter_context(tc.tile_pool(name="const", bufs=1))
        zp = ctx.enter_context(tc.tile_pool(name="zp", bufs=1, space="PSUM"))

        # fully resident SBUF tensors (nothing recycles)
        mask_full = const.tile([P, T, CP], f8, name="mask_full")
        featx_full = const.tile([P, T, FP], f8, name="featx_full")
        centw = const.tile([P, 2, C], bf16, name="centw")
        cento = const.tile([2, C], f32, name="cento")
        acc = const.tile([P, 6], f32, name="acc")
        junk = const.tile([P, H0], bf16, name="junk")
        obs = const.tile([1, 2], f32, name="obs")
        zw = const.tile([P, 128], f8, name="zw")

        # 7 PSUM banks: 4 feature-chunk accumulators + 2 ones/fsq rows
        # + 1 warmup scratch.  One tile PER BANK so Tile's dependency
        # tracking pins each epilogue STT to exactly its bank's stop-MM.
        zb = [zp.tile([P, 512], f32, name=f"zb{k}") for k in range(6)]
        psw = zp.tile([P, 512], f32, name="psw")

        nc.vector.memset(acc, 0.0)
        nc.vector.memset(zw, 0.0)

        def mask_dma(a, b):
            nc.sync.dma_start(out=mask_full[:, a:b, :],
                              in_=mask_d[:, a:b, :])

        def featx_dma(a, b):
            nc.sync.dma_start(out=featx_full[:, a:b, :],
                              in_=featx_d[:, a:b, :])

        # One FIFO HWDGE ring, just-in-time interleave: each 4-tile featx
        # chunk lands right before the two mask rounds that consume it, so
        # featx never starves the mask stream and mask tiles never wait
        # behind bulk featx.  Epilogue weights ride mid-stream when the
        # DMA queue has slack.  (Walrus encodes ONE sync wait per
        # instruction, so everything stays on one ring with FIFO-implied
        # ordering.)
        for t in range(0, T, 4):
            featx_dma(t, t + 4)
            mask_dma(t, t + 2)
            mask_dma(t + 2, t + 4)
            if t == 24:
                nc.sync.dma_start(out=centw, in_=centw_d[:, :, :])
                nc.sync.dma_start(out=cento, in_=cento_d[:, :])
        # chained 1-element DVE reads: DVE observes the epilogue-weight
        # DMAs here, so each epilogue STT later needs only its PE wait
        # (walrus encodes a limited number of sync waits per STT).
        nc.vector.tensor_copy(out=obs[0:1, 0:1], in_=centw[0:1, 0, 0:1])
        nc.vector.tensor_copy(out=obs[0:1, 1:2], in_=cento[0:1, 0:1])

        # warmup matmuls on zeros: trip the PE HAM clock gate (~3.4 µs of
        # sustained activity) during the startup DMA window so the real
        # stream runs at 2.4 GHz from its first tile.
        for _ in range(NWARM):
            nc.tensor.matmul(psw[:, 0:128], lhsT=zw[:, 0:128],
                             rhs=zw[:, 0:128], start=True, stop=True)

        # Main stream: per-bank-group staggered tile order.  Group A
        # (feature cols 0:128) processes double-tile r in round r, group B
        # (cols 128:256) tile r-2, group O (ones/fsq) tile r-4.  Each
        # group's accumulation therefore STOPS two rounds apart, letting
        # the epilogue STTs pipeline with the thinning matmul tail instead
        # of serializing after it (and the thin early rounds ease the
        # startup DMA race).
        def mm(bank, j, lhs_lo, lhs_hi, half):
            lhs = featx_full[:, 2 * j:2 * j + 2, lhs_lo:lhs_hi]
            if half == 0:
                rhs = mask_full[:, 2 * j:2 * j + 2, 0:H0]
            else:
                rhs = mask_full[:, 2 * j:2 * j + 2, H0:C]
            n = H0 if half == 0 else H1
            if bank < 4:
                out = zb[bank][:, 0:n]
            else:
                out = zb[bank][0:2, 0:n]
            nc.tensor.matmul(out, lhsT=lhs, rhs=rhs,
                             start=(j == 0), stop=(j == TD - 1),
                             perf_mode=DR)

        for r in range(TD + 4):
            if r < TD:
                mm(0, r, 0, 128, 0)
                mm(1, r, 0, 128, 1)
            if 0 <= r - 2 < TD:
                mm(2, r - 2, 128, 256, 0)
                mm(3, r - 2, 128, 256, 1)
            if 0 <= r - 4 < TD:
                mm(4, r - 4, 256, 258, 0)
                mm(5, r - 4, 256, 258, 1)

        # ---- epilogue: fused mul+reduce of Z2 against the staged W ----
        def stt(i0, i1, slot, n, parts=P):
            nc.vector.scalar_tensor_tensor(
                out=junk[0:parts, 0:n],
                in0=i0,
                scalar=1.0,
                in1=i1,
                op0=bypass,
                op1=mult,
                accum_out=acc[0:parts, slot:slot + 1],
            )

        stt(zb[0][:, 0:H0], centw[:, 0, 0:H0], 0, H0)
        stt(zb[1][:, 0:H1], centw[:, 0, H0:C], 1, H1)
        stt(zb[2][:, 0:H0], centw[:, 1, 0:H0], 2, H0)
        stt(zb[3][:, 0:H1], centw[:, 1, H0:C], 3, H1)
        stt(zb[4][0:2, 0:H0], cento[0:2, 0:H0], 4, H0, parts=2)
        stt(zb[5][0:2, 0:H1], cento[0:2, H0:C], 5, H1, parts=2)

        nc.sync.dma_start(out=out_d[:, :], in_=acc)

    if fix_waits:
        _fix_sync_waits(nc)
    return nc


def _fix_sync_waits(nc):
    """Strip provably-redundant same-engine semaphore self-waits.

    Tile encodes some cross-instruction deps as waits on the instruction's
    own engine semaphore at a value already reached by an EARLIER
    instruction on the same (in-order) engine — trivially satisfied by
    program order.  Walrus can only encode one sync wait on an STT, so
    these must go.  Every remaining compute instruction must have <=1
    wait (drains may keep several; walrus accepts that).
    """
    insts = []
    for f in nc.m.functions:
        for b in f.blocks:
            insts.extend(b.instructions)

    # which engines increment each semaphore
    updaters = {}
    out_sems = set()
    for inst in insts:
        si = inst.sync_info
        if si is None:
            continue
        for u in si.on_update:
            updaters.setdefault(u.ant_name, set()).add(inst.engine)
        if (type(inst).__name__ == "InstDMACopy" and inst.outs
                and str(inst.outs[0].memsetref).startswith("partial")):
            for u in si.on_update:
                out_sems.add(u.ant_name)
    assert out_sems, "no output DMA found"

    # cumulative per-engine increments in program order
    cum = {}
    for inst in insts:
        si = inst.sync_info
        tn = type(inst).__name__
        if si is None:
            continue
        if tn == "InstDrain" and len(si.on_wait) > 1:
            # kernel-tail drains only need the DRAM-output DMA's sem:
            # every input DMA's completion is implied by its consumers,
            # which the per-engine drains already order after.
            keep = [w for w in si.on_wait if w.ant_name in out_sems]
            assert keep, (
                f"drain {inst.name}: no output-DMA wait among "
                f"{[w.ant_name for w in si.on_wait]}")
            inst.sync_info = type(si)(on_wait=keep, on_update=si.on_update)
            si = inst.sync_info
        elif tn != "InstDrain" and len(si.on_wait) > 1:
            keep = []
            for w in si.on_wait:
                eng_cnt = cum.get((inst.engine, w.ant_name), 0)
                same_engine_only = updaters.get(w.ant_name) == {inst.engine}
                if (same_engine_only and w.wait_value is not None
                        and eng_cnt >= w.wait_value):
                    continue  # satisfied by in-order execution
                keep.append(w)
            max_waits = 2 if tn in ("InstDMACopy", "InstMatmult") else 1
            assert len(keep) <= max_waits, (
                f"{tn} {inst.name} ({inst.engine}): still multi-wait "
                f"{[(w.ant_name, w.wait_value) for w in keep]}")
            inst.sync_info = type(si)(on_wait=keep, on_update=si.on_update)
            si = inst.sync_info
        for u in si.on_update:
            key = (inst.engine, u.ant_name)
            cum[key] = cum.get(key, 0) + (u.update_value or 1)


def _shard_inputs(inputs):
    import ml_dtypes

    fp8 = ml_dtypes.float8_e4m3
    gt = np.asarray(inputs["gt"])
    features = np.asarray(inputs["features"], dtype=np.float32)
    centers = np.asarray(inputs["centers"], dtype=np.float32)

    # mask: fp8 bytes, exactly 0.0 / 1.0 (0x00 / 0x38), laid out
    # [core, p, t, c] with the class dim zero-padded to CP.
    m8 = (gt != 0).astype(np.uint8) * np.uint8(0x38)
    mask_st = np.zeros((NCORES, P, T, CP), dtype=np.uint8)
    mask_st[..., :C] = m8.reshape(NCORES, T, P, C).transpose(0, 2, 1, 3)
    mask_st = mask_st.view(fp8)

    # featx: [features | 1 | (fsq-256)/2] in fp8, same layout, padded to FP
    fsq = (features.astype(np.float64) ** 2).sum(axis=1)
    fx = np.empty((N_TOTAL, FS), dtype=np.float32)
    fx[:, 0:F] = features
    fx[:, F] = 1.0
    fx[:, F + 1] = (fsq - 256.0) * 0.5
    fx8 = fx.astype(fp8)
    featx_st = np.zeros((NCORES, P, T, FP), dtype=fp8)
    featx_st[..., :FS] = fx8.reshape(NCORES, T, P, FS).transpose(0, 2, 1, 3)

    # epilogue weights (replicated): centw[p, k, c] = -2*centers[c, 128k+p]
    centw = np.ascontiguousarray(
        (-2.0 * centers.T).reshape(2, P, C).transpose(1, 0, 2)
    ).astype(ml_dtypes.bfloat16)
    csq = (centers.astype(np.float64) ** 2).sum(axis=1)
    cento = np.empty((2, C), dtype=np.float32)
    cento[0] = csq + 256.0
    cento[1] = 2.0

    in_maps = []
    for c in range(NCORES):
        in_maps.append({
            "mask": np.ascontiguousarray(mask_st[c]),
            "featx": np.ascontiguousarray(featx_st[c]),
            "centw": centw,
            "cento": cento,
        })
    return in_maps


def _combine(results):
    """Host-side scalar combine (the all-reduce of the sharding hint).

    Per-core partial [128, 6]: cols 0-3 are per-partition sums of
    Z2_feat ∘ (-2 centers^T) (= -2·t3), cols 4-5 are valid on partitions
    0-1 only: colcnt·(csq+256) and 2·fsq-resid sums (= t1 + t2).
    """
    total = 0.0
    for r in results:
        part = np.asarray(r["partial"], dtype=np.float64)
        total += part[:, 0:4].sum() + part[0:2, 4:6].sum()
    return total / N_TOTAL


def run_spmd(inputs, trace=False):
    """Compile + run on all 8 cores. Returns (loss_scalar, BassKernelResults)."""
    from concourse.bass_utils import run_bass_kernel_spmd

    nc = build_bass()
    in_maps = _shard_inputs(inputs)
    res = run_bass_kernel_spmd(
        nc, in_maps, core_ids=list(range(NCORES)), trace=trace,
    )
    loss = _combine(res.results)
    return np.array(np.float32(loss), dtype=np.float32), res


def kernel(**inputs):
    loss, _ = run_spmd(inputs, trace=False)
    return loss


if __name__ == "__main__":
    # quick CoreSim numerical check on core 0's shard
    from concourse.bass_interp import CoreSim

    rng = np.random.default_rng(0)
    gt = (rng.integers(0, 2, size=(N_TOTAL, C))).astype(np.int32)
    features = rng.standard_normal((N_TOTAL, F)).astype(np.float32)
    centers = rng.standard_normal((C, F)).astype(np.float32)

    in_maps = _shard_inputs({"gt": gt, "features": features,
                             "centers": centers})

    nc = build_bass(fix_waits=False)
    sim = CoreSim(nc, require_finite=True, require_nnan=True)
    for k, v in in_maps[0].items():
        sim.tensor(k)[:] = v
    sim.simulate()

    got = _combine([{"partial": np.asarray(sim.tensor("partial"))}]) * N_TOTAL

    sl = slice(0, NSH)
    mask = (gt[sl] > 0).astype(np.float64)
    f64 = features[sl].astype(np.float64)
    c64 = centers.astype(np.float64)
    dist = (
        (f64 * f64).sum(1)[:, None]
        + (c64 * c64).sum(1)[None, :]
        - 2.0 * (f64 @ c64.T)
    )
    want = float((mask * dist).sum())
    print(f"sim partial sum = {got:.6e}  want = {want:.6e}  "
          f"rel = {abs(got - want) / abs(want):.3e}")


# revision 21
# speedup vs baseline: 1.1290x; 1.0163x over previous
"""CenterLoss (segment_reduce) Trainium2 Bass kernel — fp8 DoubleRow rewrite.

loss·N = t1 + t2 - 2·t3 with
  t1 = sum_i fsq[i]·rowcnt[i],  t2 = sum_c csq[c]·colcnt[c],
  t3 = sum_{c,f} Z[c,f]·centers[c,f],  Z = mask^T @ features.

Everything is folded into ONE device contraction Z2 = X^T @ mask with the
augmented X = [features | 1 | (fsq-256)/2] (258 cols, fp8) and an epilogue
elementwise reduce against W = [-2·centers^T ; csq+256 ; 2]:
  sum(Z2 ∘ W) = -2·t3 + (t2 + 256·T0) + (t1 - 256·T0) = N·loss_partial.
(The fsq column is mean-centred so its fp8 quantisation error is ~0.8% of
fsq instead of ~6%; the 256·T0 cross-terms cancel via the csq+256 row.)

Per core (8-way data-parallel on rows):
  - mask staged host-side as fp8 bytes (0.0/1.0 exact): 1 B/elt instead of
    the baseline's 4 B/elt int32 — 4x less HBM traffic on the dominant
    stream (8.26 MB/core vs 32.8 MB).
  - X is the STATIONARY matmul operand (3 chunks of 128|128|2 cols,
    weights reused across class halves so LDWEIGHTS hides), the mask
    STREAMS through the PE.  perf_mode=DoubleRow contracts 256 rows per
    pass (2 fp8 MACs/cell/cycle) — 32 double-tiles of 6 matmuls each,
    accumulating in 6 PSUM banks (class halves 512|488 to stay within the
    2 KB bank limit).
  - Epilogue: 6 DVE scalar_tensor_tensor mult+accum ops against the
    staged W -> one [128, 6] partial per core; host sums in f64 and
    divides by N (the all-reduce of the sharding hint).
All tensors are staged host-side in the exact [partition, tile, col] SBUF
layout, so every DMA is a contiguous per-partition HWDGE copy (no casts,
no rearrange descriptors).
"""

import numpy as np

N_TOTAL = 65536
C = 1000
F = 256
NCORES = 8
NSH = N_TOTAL // NCORES  # 8192 rows per core
P = 128                  # partitions (rows per k-tile)
T = NSH // P             # 64 row tiles per core
TD = T // 2              # 32 DoubleRow tiles (256 rows each)
CP = 1008                # class dim padded to %16 for DR access patterns
FP = 272                 # featx col dim padded to %16 (258 used)
FS = F + 2               # used featx cols: features | ones | fsq-resid
H0, H1 = 512, C - 512    # class halves (PSUM bank = 512 f32)


NWARM = 32  # cold N=128 warmup matmuls ≈ 3.4 µs of PE activity: spans the
            # 8..11.5 µs window between PE release and first mask tile so a
            # full HAM busy-window completes right as the real stream starts


def build_bass(fix_waits=True):
    import concourse.bass as bass
    import concourse.mybir as mybir
    import concourse.tile as tile
    from contextlib import ExitStack

    f32 = mybir.dt.float32
    bf16 = mybir.dt.bfloat16
    f8 = mybir.dt.float8e4
    DR = mybir.MatmulPerfMode.DoubleRow
    bypass = mybir.AluOpType.bypass
    mult = mybir.AluOpType.mult

    nc = bass.Bass(trn_type="TRN2")
    mask_d = nc.dram_tensor("mask", [P, T, CP], f8, kind="ExternalInput")
    featx_d = nc.dram_tensor("featx", [P, T, FP], f8, kind="ExternalInput")
    centw_d = nc.dram_tensor("centw", [P, 2, C], bf16, kind="ExternalInput")
    cento_d = nc.dram_tensor("cento", [2, C], f32, kind="ExternalInput")
    out_d = nc.dram_tensor("partial", [P, 6], f32, kind="ExternalOutput")

    with tile.TileContext(nc) as tc, ExitStack() as ctx:
        const = ctx.enter_context(tc.tile_pool(name="const", bufs=1))
        zp = ctx.enter_context(tc.tile_pool(name="zp", bufs=1, space="PSUM"))

        # fully resident SBUF tensors (nothing recycles)
        mask_full = const.tile([P, T, CP], f8, name="mask_full")
        featx_full = const.tile([P, T, FP], f8, name="featx_full")
        centw = const.tile([P, 2, C], bf16, name="centw")
        cento = const.tile([2, C], f32, name="cento")
        acc = const.tile([P, 6], f32, name="acc")
        junk = const.tile([P, H0], bf16, name="junk")
        obs = const.tile([1, 2], f32, name="obs")
        zw = const.tile([P, 128], f8, name="zw")

        # 7 PSUM banks: 4 feature-chunk accumulators + 2 ones/fsq rows
        # + 1 warmup scratch.  One tile PER BANK so Tile's dependency
        # tracking pins each epilogue STT to exactly its bank's stop-MM.
        zb = [zp.tile([P, 512], f32, name=f"zb{k}") for k in range(6)]
        psw = zp.tile([P, 512], f32, name="psw")

        nc.vector.memset(acc, 0.0)
        nc.vector.memset(zw, 0.0)

        def mask_dma(a, b):
            nc.sync.dma_start(out=mask_full[:, a:b, :],
                              in_=mask_d[:, a:b, :])

        def featx_dma(a, b):
            nc.sync.dma_start(out=featx_full[:, a:b, :],
                              in_=featx_d[:, a:b, :])

        # One FIFO HWDGE ring, just-in-time interleave: each 4-tile featx
        # chunk lands right before the two mask rounds that consume it, so
        # featx never starves the mask stream and mask tiles never wait
        # behind bulk featx.  Epilogue weights ride mid-stream when the
        # DMA queue has slack.  (Walrus encodes ONE sync wait per
        # instruction, so everything stays on one ring with FIFO-implied
        # ordering.)
        featx_dma(0, 2)
        mask_dma(0, 2)
        featx_dma(2, 6)
        mask_dma(2, 4)
        mask_dma(4, 6)
        for t in range(6, T, 4):
            featx_dma(t, min(t + 4, T))
            mask_dma(t, t + 2)
            if t + 4 <= T:
                mask_dma(t + 2, t + 4)
            if t == 26:
                nc.sync.dma_start(out=centw, in_=centw_d[:, :, :])
                nc.sync.dma_start(out=cento, in_=cento_d[:, :])
        # chained 1-element DVE reads: DVE observes the epilogue-weight
        # DMAs here, so each epilogue STT later needs only its PE wait
        # (walrus encodes a limited number of sync waits per STT).
        nc.vector.tensor_copy(out=obs[0:1, 0:1], in_=centw[0:1, 0, 0:1])
        nc.vector.tensor_copy(out=obs[0:1, 1:2], in_=cento[0:1, 0:1])

        # warmup matmuls on zeros: trip the PE HAM clock gate (~3.4 µs of
        # sustained activity) during the startup DMA window so the real
        # stream runs at 2.4 GHz from its first tile.
        for _ in range(NWARM):
            nc.tensor.matmul(psw[:, 0:128], lhsT=zw[:, 0:128],
                             rhs=zw[:, 0:128], start=True, stop=True)

        # Main stream: per-bank-group staggered tile order.  Group A
        # (feature cols 0:128) processes double-tile r in round r, group B
        # (cols 128:256) tile r-2, group O (ones/fsq) tile r-4.  Each
        # group's accumulation therefore STOPS two rounds apart, letting
        # the epilogue STTs pipeline with the thinning matmul tail instead
        # of serializing after it (and the thin early rounds ease the
        # startup DMA race).
        def mm(bank, j, lhs_lo, lhs_hi, half):
            lhs = featx_full[:, 2 * j:2 * j + 2, lhs_lo:lhs_hi]
            if half == 0:
                rhs = mask_full[:, 2 * j:2 * j + 2, 0:H0]
            else:
                rhs = mask_full[:, 2 * j:2 * j + 2, H0:C]
            n = H0 if half == 0 else H1
            if bank < 4:
                out = zb[bank][:, 0:n]
            else:
                out = zb[bank][0:2, 0:n]
            nc.tensor.matmul(out, lhsT=lhs, rhs=rhs,
                             start=(j == 0), stop=(j == TD - 1),
                             perf_mode=DR)

        for r in range(TD + 6):
            if r < TD:
                mm(0, r, 0, 128, 0)
                mm(1, r, 0, 128, 1)
            if 0 <= r - 3 < TD:
                mm(2, r - 3, 128, 256, 0)
                mm(3, r - 3, 128, 256, 1)
            if 0 <= r - 6 < TD:
                mm(4, r - 6, 256, 258, 0)
                mm(5, r - 6, 256, 258, 1)

        # ---- epilogue: fused mul+reduce of Z2 against the staged W ----
        def stt(i0, i1, slot, n, parts=P):
            nc.vector.scalar_tensor_tensor(
                out=junk[0:parts, 0:n],
                in0=i0,
                scalar=1.0,
                in1=i1,
                op0=bypass,
                op1=mult,
                accum_out=acc[0:parts, slot:slot + 1],
            )

        stt(zb[0][:, 0:H0], centw[:, 0, 0:H0], 0, H0)
        stt(zb[1][:, 0:H1], centw[:, 0, H0:C], 1, H1)
        stt(zb[2][:, 0:H0], centw[:, 1, 0:H0], 2, H0)
        stt(zb[3][:, 0:H1], centw[:, 1, H0:C], 3, H1)
        stt(zb[4][0:2, 0:H0], cento[0:2, 0:H0], 4, H0, parts=2)
        stt(zb[5][0:2, 0:H1], cento[0:2, H0:C], 5, H1, parts=2)

        nc.sync.dma_start(out=out_d[:, :], in_=acc)

    if fix_waits:
        _fix_sync_waits(nc)
    return nc


def _fix_sync_waits(nc):
    """Strip provably-redundant same-engine semaphore self-waits.

    Tile encodes some cross-instruction deps as waits on the instruction's
    own engine semaphore at a value already reached by an EARLIER
    instruction on the same (in-order) engine — trivially satisfied by
    program order.  Walrus can only encode one sync wait on an STT, so
    these must go.  Every remaining compute instruction must have <=1
    wait (drains may keep several; walrus accepts that).
    """
    insts = []
    for f in nc.m.functions:
        for b in f.blocks:
            insts.extend(b.instructions)

    # which engines increment each semaphore
    updaters = {}
    out_sems = set()
    for inst in insts:
        si = inst.sync_info
        if si is None:
            continue
        for u in si.on_update:
            updaters.setdefault(u.ant_name, set()).add(inst.engine)
        if (type(inst).__name__ == "InstDMACopy" and inst.outs
                and str(inst.outs[0].memsetref).startswith("partial")):
            for u in si.on_update:
                out_sems.add(u.ant_name)
    assert out_sems, "no output DMA found"

    # cumulative per-engine increments in program order
    cum = {}
    for inst in insts:
        si = inst.sync_info
        tn = type(inst).__name__
        if si is None:
            continue
        if tn == "InstDrain" and len(si.on_wait) > 1:
            # kernel-tail drains only need the DRAM-output DMA's sem:
            # every input DMA's completion is implied by its consumers,
            # which the per-engine drains already order after.
            keep = [w for w in si.on_wait if w.ant_name in out_sems]
            assert keep, (
                f"drain {inst.name}: no output-DMA wait among "
                f"{[w.ant_name for w in si.on_wait]}")
            inst.sync_info = type(si)(on_wait=keep, on_update=si.on_update)
            si = inst.sync_info
        elif tn != "InstDrain" and len(si.on_wait) > 1:
            keep = []
            for w in si.on_wait:
                eng_cnt = cum.get((inst.engine, w.ant_name), 0)
                same_engine_only = updaters.get(w.ant_name) == {inst.engine}
                if (same_engine_only and w.wait_value is not None
                        and eng_cnt >= w.wait_value):
                    continue  # satisfied by in-order execution
                keep.append(w)
            max_waits = 2 if tn in ("InstDMACopy", "InstMatmult") else 1
            assert len(keep) <= max_waits, (
                f"{tn} {inst.name} ({inst.engine}): still multi-wait "
                f"{[(w.ant_name, w.wait_value) for w in keep]}")
            inst.sync_info = type(si)(on_wait=keep, on_update=si.on_update)
            si = inst.sync_info
        for u in si.on_update:
            key = (inst.engine, u.ant_name)
            cum[key] = cum.get(key, 0) + (u.update_value or 1)


def _shard_inputs(inputs):
    import ml_dtypes

    fp8 = ml_dtypes.float8_e4m3
    gt = np.asarray(inputs["gt"])
    features = np.asarray(inputs["features"], dtype=np.float32)
    centers = np.asarray(inputs["centers"], dtype=np.float32)

    # mask: fp8 bytes, exactly 0.0 / 1.0 (0x00 / 0x38), laid out
    # [core, p, t, c] with the class dim zero-padded to CP.
    m8 = (gt != 0).astype(np.uint8) * np.uint8(0x38)
    mask_st = np.zeros((NCORES, P, T, CP), dtype=np.uint8)
    mask_st[..., :C] = m8.reshape(NCORES, T, P, C).transpose(0, 2, 1, 3)
    mask_st = mask_st.view(fp8)

    # featx: [features | 1 | (fsq-256)/2] in fp8, same layout, padded to FP
    fsq = (features.astype(np.float64) ** 2).sum(axis=1)
    fx = np.empty((N_TOTAL, FS), dtype=np.float32)
    fx[:, 0:F] = features
    fx[:, F] = 1.0
    fx[:, F + 1] = (fsq - 256.0) * 0.5
    fx8 = fx.astype(fp8)
    featx_st = np.zeros((NCORES, P, T, FP), dtype=fp8)
    featx_st[..., :FS] = fx8.reshape(NCORES, T, P, FS).transpose(0, 2, 1, 3)

    # epilogue weights (replicated): centw[p, k, c] = -2*centers[c, 128k+p]
    centw = np.ascontiguousarray(
        (-2.0 * centers.T).reshape(2, P, C).transpose(1, 0, 2)
    ).astype(ml_dtypes.bfloat16)
    csq = (centers.astype(np.float64) ** 2).sum(axis=1)
    cento = np.empty((2, C), dtype=np.float32)
    cento[0] = csq + 256.0
    cento[1] = 2.0

    in_maps = []
    for c in range(NCORES):
        in_maps.append({
            "mask": np.ascontiguousarray(mask_st[c]),
            "featx": np.ascontiguousarray(featx_st[c]),
            "centw": centw,
            "cento": cento,
        })
    return in_maps


def _combine(results):
    """Host-side scalar combine (the all-reduce of the sharding hint).

    Per-core partial [128, 6]: cols 0-3 are per-partition sums of
    Z2_feat ∘ (-2 centers^T) (= -2·t3), cols 4-5 are valid on partitions
    0-1 only: colcnt·(csq+256) and 2·fsq-resid sums (= t1 + t2).
    """
    total = 0.0
    for r in results:
        part = np.asarray(r["partial"], dtype=np.float64)
        total += part[:, 0:4].sum() + part[0:2, 4:6].sum()
    return total / N_TOTAL


def run_spmd(inputs, trace=False):
    """Compile + run on all 8 cores. Returns (loss_scalar, BassKernelResults)."""
    from concourse.bass_utils import run_bass_kernel_spmd

    nc = build_bass()
    in_maps = _shard_inputs(inputs)
    res = run_bass_kernel_spmd(
        nc, in_maps, core_ids=list(range(NCORES)), trace=trace,
    )
    loss = _combine(res.results)
    return np.array(np.float32(loss), dtype=np.float32), res


def kernel(**inputs):
    loss, _ = run_spmd(inputs, trace=False)
    return loss


if __name__ == "__main__":
    # quick CoreSim numerical check on core 0's shard
    from concourse.bass_interp import CoreSim

    rng = np.random.default_rng(0)
    gt = (rng.integers(0, 2, size=(N_TOTAL, C))).astype(np.int32)
    features = rng.standard_normal((N_TOTAL, F)).astype(np.float32)
    centers = rng.standard_normal((C, F)).astype(np.float32)

    in_maps = _shard_inputs({"gt": gt, "features": features,
                             "centers": centers})

    nc = build_bass(fix_waits=False)
    sim = CoreSim(nc, require_finite=True, require_nnan=True)
    for k, v in in_maps[0].items():
        sim.tensor(k)[:] = v
    sim.simulate()

    got = _combine([{"partial": np.asarray(sim.tensor("partial"))}]) * N_TOTAL

    sl = slice(0, NSH)
    mask = (gt[sl] > 0).astype(np.float64)
    f64 = features[sl].astype(np.float64)
    c64 = centers.astype(np.float64)
    dist = (
        (f64 * f64).sum(1)[:, None]
        + (c64 * c64).sum(1)[None, :]
        - 2.0 * (f64 @ c64.T)
    )
    want = float((mask * dist).sum())
    print(f"sim partial sum = {got:.6e}  want = {want:.6e}  "
          f"rel = {abs(got - want) / abs(want):.3e}")
